# revision 5
# baseline (speedup 1.0000x reference)
"""Trainium2 Bass kernel for the LSH MoE router (8-core SPMD).

Strategy (token/data parallel, per sharding hint):
  - x [2,2048,2048] flattens to [4096, 2048] tokens; each of the 8 cores
    owns 512 contiguous tokens (4 blocks of 128).
  - Per 128-token block: DMA x block, PE-transpose 128x128 chunks,
    fp32 matmul with replicated W -> logits [128, 8] in PSUM.
  - top-2 via DVE max/max_index; gates via exp/reciprocal (softmax of 2).
  - one-hot (k,e) masks [128, 16]; block-level cumsum via triangular-ones
    matmul on the PE; cross-core prefix of per-core counts [1,16] via an
    AllGather collective + per-core 0/1 weight vector matmul.
  - dispatcher row one-hots are built by comparing an int16 iota (1..4096)
    against per-token scalar s = e*512 + p_global (0 when over capacity),
    written as bf16 (values 0/1 exact) to halve the HBM write traffic;
    host casts back to f32.
"""
import sys
sys.path.insert(0, '/opt/trn_rl_repo')
import numpy as np

N_CORES = 8
B, S, D = 2, 2048, 2048
N = B * S          # 4096 tokens
E = 8              # experts
K = 2              # top-k
CAP = N // E       # 512 capacity
TOK = N // N_CORES # 512 tokens per core
NB = TOK // 128    # 4 blocks
F = E * CAP        # 4096 dispatcher row length
DC = D // 128      # 16 contraction chunks

_CACHE = {}


def _emit_body(nc, tc, mybir, pools, tiles):
    """Emit one full router iteration (phases 1, collective, 2)."""
    f32 = mybir.dt.float32
    AX = mybir.AxisListType.X
    OP = mybir.AluOpType
    AF = mybir.ActivationFunctionType

    (xpool, xtpool, spool, kpool, dpool, ps_t, ps_lg, ps_pr, ps_sm) = pools
    (xs, dispb, gates, inds, cc_in, cc_out,
     UT, ONES, IDN, wv, er, wsb, iota1, iota8, offblock) = tiles

    masks = []
    ikeeps = []
    # ---- phase 1: logits, top-2, gates, masks ----
    for b in range(NB):
        xt = xpool.tile([128, D], f32, tag="xt")
        nc.sync.dma_start(xt[:], xs[b * 128:(b + 1) * 128, :])
        lg_ps = ps_lg.tile([128, 8], f32, tag="lg")
        for c in range(DC):
            xT_ps = ps_t.tile([128, 128], f32, tag="xT")
            nc.tensor.transpose(xT_ps[:], xt[:, c * 128:(c + 1) * 128], IDN)
            xTsb = xtpool.tile([128, 128], f32, tag="xTsb")
            nc.scalar.copy(xTsb[:], xT_ps[:])
            nc.tensor.matmul(lg_ps[:], xTsb[:], wsb[:, c * E:(c + 1) * E],
                             start=(c == 0), stop=(c == DC - 1))
        lg = spool.tile([128, 8], f32, tag="lg_sb")
        nc.scalar.copy(lg[:], lg_ps[:])
        topv = spool.tile([128, 8], f32, tag="topv")
        topi = spool.tile([128, 8], mybir.dt.uint32, tag="topi")
        nc.vector.max(topv[:], lg[:])
        nc.vector.max_index(topi[:], topv[:], lg[:])

        ik = kpool.tile([128, 2], f32, tag=f"ik{b}")
        nc.vector.tensor_copy(ik[:], topi[:, 0:2])
        ii = spool.tile([128, 2], mybir.dt.int32, tag="ii")
        nc.vector.tensor_copy(ii[:], topi[:, 0:2])
        nc.sync.dma_start(inds[b * 128:(b + 1) * 128, :], ii[:])

        dgl = spool.tile([128, 1], f32, tag="dgl")
        nc.vector.tensor_sub(dgl[:], topv[:, 1:2], topv[:, 0:1])
        ex = spool.tile([128, 1], f32, tag="ex")
        nc.scalar.activation(ex[:], dgl[:], AF.Exp)
        sm = spool.tile([128, 1], f32, tag="sm")
        nc.vector.tensor_scalar_add(sm[:], ex[:], 1.0)
        gsb = spool.tile([128, 2], f32, tag="gsb")
        nc.vector.reciprocal(gsb[:, 0:1], sm[:])
        nc.vector.tensor_mul(gsb[:, 1:2], ex[:], gsb[:, 0:1])
        nc.sync.dma_start(gates[b * 128:(b + 1) * 128, :], gsb[:])

        mk = kpool.tile([128, 16], f32, tag=f"mk{b}")
        nc.vector.tensor_scalar(mk[:, 0:8], iota8[:], ik[:, 0:1],
                                None, op0=OP.is_equal)
        nc.vector.tensor_scalar(mk[:, 8:16], iota8[:], ik[:, 1:2],
                                None, op0=OP.is_equal)
        masks.append(mk)
        ikeeps.append(ik)

    # ---- cross-core prefix: counts -> AllGather -> offsets ----
    counts_ps = ps_sm.tile([1, 16], f32, tag="counts_ps")
    for b in range(NB):
        nc.tensor.matmul(counts_ps[:], ONES[:, 0:1], masks[b][:],
                         start=(b == 0), stop=(b == NB - 1))
    counts_sb = spool.tile([1, 16], f32, tag="counts")
    nc.vector.tensor_copy(counts_sb[:], counts_ps[:])
    nc.sync.dma_start(cc_in.ap()[:], counts_sb[:])
    OPc = mybir.AluOpType
    nc.gpsimd.collective_compute(
        "AllGather", OPc.bypass,
        replica_groups=[list(range(N_CORES))],
        ins=[cc_in.ap().opt()],
        outs=[cc_out.ap().opt()],
    )
    ag = spool.tile([8, 16], f32, tag="ag")
    nc.sync.dma_start(ag[:], cc_out.ap()[:])
    off_ps = ps_sm.tile([1, 16], f32, tag="off_ps")
    nc.tensor.matmul(off_ps[:], wv[:], ag[:], start=True, stop=True)
    offrow = spool.tile([1, 16], f32, tag="offrow")
    nc.vector.tensor_add(offrow[:], off_ps[:], er[:])
    nc.vector.tensor_copy(offblock[0:1, :], offrow[:])

    # ---- phase 2: priorities, dispatch one-hots ----
    for b in range(NB):
        pr_ps = ps_pr.tile([128, 16], f32, tag="pr")
        nc.tensor.matmul(pr_ps[:], UT, masks[b][:], start=True, stop=False)
        for bp in range(b):
            nc.tensor.matmul(pr_ps[:], ONES, masks[bp][:],
                             start=False, stop=False)
        nc.tensor.matmul(pr_ps[:], ONES, offblock[:], start=False, stop=True)
        ss = []
        for k in range(K):
            tmp8 = spool.tile([128, 8], f32, tag="tmp8")
            nc.vector.tensor_mul(tmp8[:], pr_ps[:, k * 8:(k + 1) * 8],
                                 masks[b][:, k * 8:(k + 1) * 8])
            sel = spool.tile([128, 1], f32, tag="sel")
            nc.vector.tensor_reduce(sel[:], tmp8[:], axis=AX, op=OP.add)
            tcap = spool.tile([128, 1], f32, tag="tcap")
            nc.scalar.activation(tcap[:], ikeeps[b][:, k:k + 1],
                                 AF.Copy, scale=512.0, bias=512.0)
            vld = spool.tile([128, 1], f32, tag="vld")
            nc.vector.tensor_scalar(vld[:], sel[:], tcap[:, 0:1],
                                    None, op0=OP.is_le)
            s_k = spool.tile([128, 1], f32, tag=f"s{k}")
            nc.vector.tensor_mul(s_k[:], sel[:], vld[:])
            ss.append(s_k)
        eq = dpool.tile([128, F], mybir.dt.bfloat16, tag="eq")
        nc.vector.tensor_scalar(eq[:], iota1[:], ss[0][:, 0:1],
                                None, op0=OP.is_equal)
        dt_ = dpool.tile([128, F], mybir.dt.bfloat16, tag="disp")
        nc.vector.scalar_tensor_tensor(dt_[:], iota1[:], ss[1][:, 0:1],
                                       eq[:], op0=OP.is_equal, op1=OP.add)
        nc.sync.dma_start(dispb[b * 128:(b + 1) * 128, :], dt_[:])


def _build(repeat: int = 1):
    from concourse import bacc, mybir, tile

    f32 = mybir.dt.float32
    nc = bacc.Bacc("TRN2", target_bir_lowering=False, debug=False,
                   num_devices=N_CORES)

    xs = nc.dram_tensor("xs", [TOK, D], f32, kind="ExternalInput").ap()
    w_in = nc.dram_tensor("w", [D, E], f32, kind="ExternalInput").ap()
    consts = nc.dram_tensor("consts", [128, 3 * 128], f32,
                            kind="ExternalInput").ap()  # U | ONES | IDN
    wvec_in = nc.dram_tensor("wvec", [8, 1], f32, kind="ExternalInput").ap()
    erow_in = nc.dram_tensor("erow", [1, 16], f32, kind="ExternalInput").ap()

    dispb = nc.dram_tensor("dispb", [TOK, F], mybir.dt.bfloat16,
                           kind="ExternalOutput").ap()
    gates = nc.dram_tensor("gates", [TOK, K], f32, kind="ExternalOutput").ap()
    inds = nc.dram_tensor("inds", [TOK, K], mybir.dt.int32,
                          kind="ExternalOutput").ap()

    cc_in = nc.dram_tensor("cc_in", [1, 16], f32)
    cc_out = nc.dram_tensor("cc_out", [8, 16], f32, addr_space="Shared")

    with tile.TileContext(nc) as tc:
        with (
            tc.tile_pool(name="const", bufs=1) as cpool,
            tc.tile_pool(name="xin", bufs=2) as xpool,
            tc.tile_pool(name="xt", bufs=3) as xtpool,
            tc.tile_pool(name="small", bufs=6) as spool,
            tc.tile_pool(name="keep", bufs=1) as kpool,
            tc.tile_pool(name="disp", bufs=3) as dpool,
            tc.tile_pool(name="ps_t", bufs=2, space="PSUM") as ps_t,
            tc.tile_pool(name="ps_lg", bufs=2, space="PSUM") as ps_lg,
            tc.tile_pool(name="ps_pr", bufs=2, space="PSUM") as ps_pr,
            tc.tile_pool(name="ps_sm", bufs=1, space="PSUM") as ps_sm,
        ):
            cons = cpool.tile([128, 3 * 128], f32)
            nc.sync.dma_start(cons[:], consts[:])
            UT = cons[:, 0:128]
            ONES = cons[:, 128:256]
            IDN = cons[:, 256:384]

            wv = cpool.tile([8, 1], f32)
            nc.sync.dma_start(wv[:], wvec_in[:])
            er = cpool.tile([1, 16], f32)
            nc.sync.dma_start(er[:], erow_in[:])

            wsb = cpool.tile([128, DC * E], f32)
            nc.sync.dma_start(
                wsb[:].rearrange("p (c e) -> p c e", e=E),
                w_in.rearrange("(c p) e -> p c e", p=128),
            )

            iota1 = cpool.tile([128, F], mybir.dt.int16)
            nc.gpsimd.iota(iota1[:], pattern=[[1, F]], base=1,
                           channel_multiplier=0)
            iota8 = cpool.tile([128, 8], f32)
            nc.gpsimd.iota(iota8[:], pattern=[[1, 8]], base=0,
                           channel_multiplier=0,
                           allow_small_or_imprecise_dtypes=True)
            offblock = cpool.tile([128, 16], f32)
            nc.vector.memset(offblock[:], 0.0)

            pools = (xpool, xtpool, spool, kpool, dpool,
                     ps_t, ps_lg, ps_pr, ps_sm)
            tiles = (xs, dispb, gates, inds, cc_in, cc_out,
                     UT, ONES, IDN, wv, er, wsb, iota1, iota8, offblock)
            for _rep in range(repeat):
                _emit_body(nc, tc, mybir, pools, tiles)

    nc.compile()
    return nc


def _get_nc():
    if "nc" not in _CACHE:
        _CACHE["nc"] = _build()
    return _CACHE["nc"]


def make_in_maps(x2: np.ndarray, Wc: np.ndarray):
    UTc = np.triu(np.ones((128, 128), np.float32))  # U[j,i] = 1 for j<=i
    ONESc = np.ones((128, 128), np.float32)
    IDNc = np.eye(128, dtype=np.float32)
    consts = np.concatenate([UTc, ONESc, IDNc], axis=1)
    erow = np.tile(np.arange(E, dtype=np.float32) * CAP, K)[None, :]

    in_maps = []
    for c in range(N_CORES):
        wvec = np.zeros((8, 1), np.float32)
        wvec[:c] = 1.0
        in_maps.append({
            "xs": np.ascontiguousarray(x2[c * TOK:(c + 1) * TOK]),
            "w": Wc,
            "consts": consts,
            "wvec": wvec,
            "erow": erow,
        })
    return in_maps


def kernel(x: np.ndarray, W: np.ndarray):
    from concourse import bass_utils

    nc = _get_nc()

    x2 = np.ascontiguousarray(x.reshape(N, D), dtype=np.float32)
    Wc = np.ascontiguousarray(W, dtype=np.float32)
    in_maps = make_in_maps(x2, Wc)

    res = bass_utils.run_bass_kernel_spmd(nc, in_maps,
                                          core_ids=list(range(N_CORES)))

    disp = np.concatenate(
        [np.asarray(res.results[c]["dispb"]).astype(np.float32)
         for c in range(N_CORES)], axis=0).reshape(N, E, CAP)
    gw = np.concatenate(
        [res.results[c]["gates"] for c in range(N_CORES)],
        axis=0).reshape(B, S, K).astype(np.float32)
    ei = np.concatenate(
        [res.results[c]["inds"] for c in range(N_CORES)],
        axis=0).reshape(B, S, K).astype(np.int32)
    return disp, gw, ei


# revision 20
# speedup vs baseline: 1.2446x; 1.2446x over previous
"""Trainium2 Bass kernel for the LSH MoE router (8-core SPMD).

Strategy (token/data parallel, per sharding hint):
  - x [2,2048,2048] flattens to [4096, 2048] tokens; each of the 8 cores
    owns 512 contiguous tokens (4 blocks of 128).
  - Per 128-token block: DMA x block, PE-transpose 128x128 chunks,
    fp32 matmul with replicated W -> logits [128, 8] in PSUM.
  - top-2 via DVE max/max_index; gates via exp/reciprocal (softmax of 2).
  - one-hot (k,e) masks [128, 16]; block-level cumsum via triangular-ones
    matmul on the PE; cross-core prefix of per-core counts [1,16] via an
    AllGather collective + per-core 0/1 weight vector matmul.
  - dispatcher row one-hots are built by comparing an int16 iota (1..4096)
    against per-token scalar s = e*512 + p_global (0 when over capacity),
    written as bf16 (values 0/1 exact) to halve the HBM write traffic;
    host casts back to f32.
"""
import sys
sys.path.insert(0, '/opt/trn_rl_repo')
import numpy as np

N_CORES = 8
B, S, D = 2, 2048, 2048
N = B * S          # 4096 tokens
E = 8              # experts
K = 2              # top-k
CAP = N // E       # 512 capacity
TOK = N // N_CORES # 512 tokens per core
NB = TOK // 128    # 4 blocks
F = E * CAP        # 4096 dispatcher row length
DC = D // 128      # 16 contraction chunks

_CACHE = {}
_SKIP_EQ = False


def _emit_body(nc, tc, mybir, pools, tiles, with_cc=True):
    """Emit one full router iteration (phases 1, collective, 2)."""
    f32 = mybir.dt.float32
    AX = mybir.AxisListType.X
    OP = mybir.AluOpType
    AF = mybir.ActivationFunctionType

    (xrpool, spool, kpool, dpool, ps_lg, ps_pr, ps_sm) = pools
    (xs, dispb, gates, inds, cc_in, cc_out,
     UT, ONES, wv, er, wsb, iota1, iota8, offblock) = tiles

    # xs is the host-pre-transposed shard xT [D, TOK]; load it fully
    # resident as 16 contiguous [128, TOK] tiles (d-major groups).
    xr = []
    for c in range(DC):
        t = xrpool.tile([128, TOK], f32, tag=f"xr{c}")
        eng = nc.sync if c % 2 == 0 else nc.gpsimd
        eng.dma_start(t[:], xs[c * 128:(c + 1) * 128, :])
        xr.append(t)

    masks = []
    ikeeps = []
    # ---- phase 1: logits, top-2, gates, masks ----
    for b in range(NB):
        lg_ps = ps_lg.tile([128, 8], f32, tag="lg")
        for c in range(DC):
            nc.tensor.matmul(lg_ps[:], xr[c][:, b * 128:(b + 1) * 128],
                             wsb[:, c * E:(c + 1) * E],
                             start=(c == 0), stop=(c == DC - 1))
        lg = spool.tile([128, 8], f32, tag="lg_sb")
        nc.scalar.copy(lg[:], lg_ps[:])
        topv = spool.tile([128, 8], f32, tag="topv")
        topi = spool.tile([128, 8], mybir.dt.uint32, tag="topi")
        nc.vector.max(topv[:], lg[:])
        nc.vector.max_index(topi[:], topv[:], lg[:])

        ik = kpool.tile([128, 2], f32, tag=f"ik{b}")
        nc.vector.tensor_copy(ik[:], topi[:, 0:2])
        ii = spool.tile([128, 2], mybir.dt.int32, tag="ii")
        nc.vector.tensor_copy(ii[:], topi[:, 0:2])
        nc.sync.dma_start(inds[b * 128:(b + 1) * 128, :], ii[:])

        dgl = spool.tile([128, 1], f32, tag="dgl")
        nc.vector.tensor_sub(dgl[:], topv[:, 1:2], topv[:, 0:1])
        ex = spool.tile([128, 1], f32, tag="ex")
        nc.scalar.activation(ex[:], dgl[:], AF.Exp)
        sm = spool.tile([128, 1], f32, tag="sm")
        nc.vector.tensor_scalar_add(sm[:], ex[:], 1.0)
        gsb = spool.tile([128, 2], f32, tag="gsb")
        nc.vector.reciprocal(gsb[:, 0:1], sm[:])
        nc.vector.tensor_mul(gsb[:, 1:2], ex[:], gsb[:, 0:1])
        nc.sync.dma_start(gates[b * 128:(b + 1) * 128, :], gsb[:])

        mk = kpool.tile([128, 16], f32, tag=f"mk{b}")
        nc.vector.tensor_scalar(mk[:, 0:8], iota8[:], ik[:, 0:1],
                                None, op0=OP.is_equal)
        nc.vector.tensor_scalar(mk[:, 8:16], iota8[:], ik[:, 1:2],
                                None, op0=OP.is_equal)
        masks.append(mk)
        ikeeps.append(ik)

    # ---- cross-core prefix: counts -> AllGather -> offsets ----
    counts_ps = ps_sm.tile([1, 16], f32, tag="counts_ps")
    for b in range(NB):
        nc.tensor.matmul(counts_ps[:], ONES[:, 0:1], masks[b][:],
                         start=(b == 0), stop=(b == NB - 1))
    counts_sb = spool.tile([1, 16], f32, tag="counts")
    nc.vector.tensor_copy(counts_sb[:], counts_ps[:])
    if with_cc:
        nc.sync.dma_start(cc_in.ap()[:], counts_sb[:])
        OPc = mybir.AluOpType
        nc.gpsimd.collective_compute(
            "AllGather", OPc.bypass,
            replica_groups=[list(range(N_CORES))],
            ins=[cc_in.ap().opt()],
            outs=[cc_out.ap().opt()],
        )
        ag = spool.tile([8, 16], f32, tag="ag")
        nc.sync.dma_start(ag[:], cc_out.ap()[:])
        off_ps = ps_sm.tile([1, 16], f32, tag="off_ps")
        nc.tensor.matmul(off_ps[:], wv[:], ag[:], start=True, stop=True)
        offrow = spool.tile([1, 16], f32, tag="offrow")
        nc.vector.tensor_add(offrow[:], off_ps[:], er[:])
    else:
        offrow = spool.tile([1, 16], f32, tag="offrow")
        nc.vector.tensor_add(offrow[:], counts_sb[:], er[:])
    nc.vector.tensor_copy(offblock[0:1, :], offrow[:])

    # ---- phase 2: priorities, dispatch one-hots ----
    for b in range(NB):
        pr_ps = ps_pr.tile([128, 16], f32, tag="pr")
        nc.tensor.matmul(pr_ps[:], UT, masks[b][:], start=True, stop=False)
        for bp in range(b):
            nc.tensor.matmul(pr_ps[:], ONES, masks[bp][:],
                             start=False, stop=False)
        nc.tensor.matmul(pr_ps[:], ONES, offblock[:], start=False, stop=True)
        ss = []
        for k in range(K):
            # fused: tmp8 = pr*mask with accum_out sel = row-sum(tmp8)
            tmp8 = spool.tile([128, 8], f32, tag="tmp8")
            sel = spool.tile([128, 1], f32, tag="sel")
            nc.vector.scalar_tensor_tensor(
                tmp8[:], pr_ps[:, k * 8:(k + 1) * 8], 0.0,
                masks[b][:, k * 8:(k + 1) * 8],
                op0=OP.bypass, op1=OP.mult, accum_out=sel[:])
            tcap = spool.tile([128, 1], f32, tag="tcap")
            nc.scalar.activation(tcap[:], ikeeps[b][:, k:k + 1],
                                 AF.Copy, scale=512.0, bias=512.0)
            # fused: s_k = (sel <= tcap) * sel
            s_k = spool.tile([128, 1], f32, tag=f"s{k}")
            nc.vector.scalar_tensor_tensor(
                s_k[:], sel[:], tcap[:, 0:1], sel[:],
                op0=OP.is_le, op1=OP.mult)
            ss.append(s_k)
        if _SKIP_EQ:
            continue
        eq = dpool.tile([128, F], mybir.dt.bfloat16, tag="eq")
        nc.vector.tensor_scalar(eq[:], iota1[:], ss[0][:, 0:1],
                                None, op0=OP.is_equal)
        dt_ = dpool.tile([128, F], mybir.dt.bfloat16, tag="disp")
        nc.vector.scalar_tensor_tensor(dt_[:], iota1[:], ss[1][:, 0:1],
                                       eq[:], op0=OP.is_equal, op1=OP.add)
        eng = nc.sync if b % 2 == 0 else nc.gpsimd
        eng.dma_start(dispb[b * 128:(b + 1) * 128, :], dt_[:])


def _build(repeat: int = 1, with_cc: bool = True, num_devices: int = N_CORES):
    from concourse import bacc, mybir, tile

    f32 = mybir.dt.float32
    nc = bacc.Bacc("TRN2", target_bir_lowering=False, debug=False,
                   num_devices=num_devices)

    xs = nc.dram_tensor("xs", [D, TOK], f32, kind="ExternalInput").ap()
    w_in = nc.dram_tensor("w", [D, E], f32, kind="ExternalInput").ap()
    consts = nc.dram_tensor("consts", [128, 2 * 128], f32,
                            kind="ExternalInput").ap()  # U | ONES
    wvec_in = nc.dram_tensor("wvec", [8, 1], f32, kind="ExternalInput").ap()
    erow_in = nc.dram_tensor("erow", [1, 16], f32, kind="ExternalInput").ap()

    dispb = nc.dram_tensor("dispb", [TOK, F], mybir.dt.bfloat16,
                           kind="ExternalOutput").ap()
    gates = nc.dram_tensor("gates", [TOK, K], f32, kind="ExternalOutput").ap()
    inds = nc.dram_tensor("inds", [TOK, K], mybir.dt.int32,
                          kind="ExternalOutput").ap()

    cc_in = nc.dram_tensor("cc_in", [1, 16], f32)
    cc_out = nc.dram_tensor("cc_out", [8, 16], f32, addr_space="Shared")

    with tile.TileContext(nc) as tc:
        with (
            tc.tile_pool(name="const", bufs=1) as cpool,
            tc.tile_pool(name="xres", bufs=1) as xrpool,
            tc.tile_pool(name="small", bufs=6) as spool,
            tc.tile_pool(name="keep", bufs=1) as kpool,
            tc.tile_pool(name="disp", bufs=4) as dpool,
            tc.tile_pool(name="ps_lg", bufs=2, space="PSUM") as ps_lg,
            tc.tile_pool(name="ps_pr", bufs=2, space="PSUM") as ps_pr,
            tc.tile_pool(name="ps_sm", bufs=1, space="PSUM") as ps_sm,
        ):
            cons = cpool.tile([128, 2 * 128], f32)
            nc.sync.dma_start(cons[:], consts[:])
            UT = cons[:, 0:128]
            ONES = cons[:, 128:256]

            wv = cpool.tile([8, 1], f32)
            nc.sync.dma_start(wv[:], wvec_in[:])
            er = cpool.tile([1, 16], f32)
            nc.sync.dma_start(er[:], erow_in[:])

            wsb = cpool.tile([128, DC * E], f32)
            nc.sync.dma_start(
                wsb[:].rearrange("p (c e) -> p c e", e=E),
                w_in.rearrange("(c p) e -> p c e", p=128),
            )

            iota1 = cpool.tile([128, F], mybir.dt.int16)
            nc.gpsimd.iota(iota1[:], pattern=[[1, F]], base=1,
                           channel_multiplier=0)
            iota8 = cpool.tile([128, 8], f32)
            nc.gpsimd.iota(iota8[:], pattern=[[1, 8]], base=0,
                           channel_multiplier=0,
                           allow_small_or_imprecise_dtypes=True)
            offblock = cpool.tile([128, 16], f32)
            nc.vector.memset(offblock[:], 0.0)

            pools = (xrpool, spool, kpool, dpool, ps_lg, ps_pr, ps_sm)
            tiles = (xs, dispb, gates, inds, cc_in, cc_out,
                     UT, ONES, wv, er, wsb, iota1, iota8, offblock)
            for _rep in range(repeat):
                _emit_body(nc, tc, mybir, pools, tiles, with_cc=with_cc)

    nc.compile()
    return nc


def _get_nc():
    if "nc" not in _CACHE:
        _CACHE["nc"] = _build()
    return _CACHE["nc"]


def make_in_maps(x2: np.ndarray, Wc: np.ndarray):
    UTc = np.triu(np.ones((128, 128), np.float32))  # U[j,i] = 1 for j<=i
    ONESc = np.ones((128, 128), np.float32)
    consts = np.concatenate([UTc, ONESc], axis=1)
    erow = np.tile(np.arange(E, dtype=np.float32) * CAP, K)[None, :]

    in_maps = []
    for c in range(N_CORES):
        wvec = np.zeros((8, 1), np.float32)
        wvec[:c] = 1.0
        in_maps.append({
            "xs": np.ascontiguousarray(x2[c * TOK:(c + 1) * TOK].T),
            "w": Wc,
            "consts": consts,
            "wvec": wvec,
            "erow": erow,
        })
    return in_maps


def kernel(x: np.ndarray, W: np.ndarray):
    from concourse import bass_utils

    nc = _get_nc()

    x2 = np.ascontiguousarray(x.reshape(N, D), dtype=np.float32)
    Wc = np.ascontiguousarray(W, dtype=np.float32)
    in_maps = make_in_maps(x2, Wc)

    res = bass_utils.run_bass_kernel_spmd(nc, in_maps,
                                          core_ids=list(range(N_CORES)))

    disp = np.concatenate(
        [np.asarray(res.results[c]["dispb"]).astype(np.float32)
         for c in range(N_CORES)], axis=0).reshape(N, E, CAP)
    gw = np.concatenate(
        [res.results[c]["gates"] for c in range(N_CORES)],
        axis=0).reshape(B, S, K).astype(np.float32)
    ei = np.concatenate(
        [res.results[c]["inds"] for c in range(N_CORES)],
        axis=0).reshape(B, S, K).astype(np.int32)
    return disp, gw, ei
